# revision 3
# baseline (speedup 1.0000x reference)
"""Causal self-attention (B=4, T=2048, C=1024, NH=16) on 8 trn2 NeuronCores.

Sharding: hybrid batch x head tensor-parallel. Core c handles batch b=c//2 and
heads [8*(c%2), 8*(c%2)+8) (feature slice of 512 of each of Q/K/V). Each core:
  - PE-transposes its x[b] slab to get x^T (contraction dim on partitions),
  - computes Q^T,K^T in bf16 [feature, token] layout and V (+bias, with an
    appended ones-column for the softmax denominator) in [token, feature],
  - runs causal attention per head-pair in score-transposed layout S^T[k,q]
    (softmax without max-subtraction: scores ~ N(0,1), no overflow risk);
    scores matmuls in bf16, exact causal widths; the diagonal-block mask is
    applied by gpsimd affine_select (keeps DVE off the critical path),
  - normalizes via: reciprocal of the denominator row, DMA of that row to
    partition 0, gpsimd partition_broadcast, and all-SBUF multiplies (no
    psum is held by normalization, so the S double-buffer never blocks),
  - applies its 512-row slice of w_proj to produce a partial output.
Host sums the two partial outputs per batch and adds b_proj.

Matmuls run as float32r (full PE rate at free dim >= 256) or bf16.
All bulk DMA rides the two HWDGE queues (sync/scalar); gpsimd DMAs execute
serially on the Pool engine and are avoided on the critical path.
"""

import sys

for _p in ("/opt/trn_rl_repo",):
    if _p not in sys.path:
        sys.path.append(_p)

import numpy as np

import concourse.bacc as bacc
import concourse.bass as bass
import concourse.mybir as mybir
from concourse.masks import make_identity
from concourse.tile import TileContext

P = 128
B, T, C, NH, HD = 4, 2048, 1024, 16, 64
NCORES = 8
FH = 512            # features per core = 8 heads * HD
NHC = 8             # heads per core
QT = 512            # q-tile width
NQT = T // QT       # 4
NKT = T // P        # 16 k-tiles
CC = C // P         # 8 contraction chunks
FC = FH // P        # 4 feature chunks
DT = mybir.dt.float32
DTR = mybir.dt.float32r   # reduced-precision matmul dtype
DTB = mybir.dt.bfloat16   # attention Q/K/P/V dtype




def _r(ap):
    """Bitcast an AP to the matmul compute dtype."""
    return ap.bitcast(DTR)


def build_nc():
    nc = bacc.Bacc("TRN2", target_bir_lowering=False, debug=False,
                   num_devices=NCORES)
    xb = nc.dram_tensor("xb", [T, C], DT, kind="ExternalInput")
    wq = nc.dram_tensor("wq", [C, FH], DT, kind="ExternalInput")
    wk = nc.dram_tensor("wk", [C, FH], DT, kind="ExternalInput")
    wv = nc.dram_tensor("wv", [C, FH], DT, kind="ExternalInput")
    wp = nc.dram_tensor("wp", [FH, C], DT, kind="ExternalInput")
    bq = nc.dram_tensor("bq", [FH], DT, kind="ExternalInput")
    bk = nc.dram_tensor("bk", [FH], DT, kind="ExternalInput")
    bv = nc.dram_tensor("bv", [FH], DT, kind="ExternalInput")
    yp = nc.dram_tensor("yp", [T, C], DT, kind="ExternalOutput")

    Exp = mybir.ActivationFunctionType.Exp

    with TileContext(nc) as tc:
        with (
            tc.tile_pool(name="const", bufs=1) as const,
            tc.tile_pool(name="big", bufs=1) as big,
            tc.tile_pool(name="wqkv", bufs=1) as wpool,
            tc.tile_pool(name="qTp", bufs=4) as qTp,
            tc.tile_pool(name="atp", bufs=2) as atp,
            tc.tile_pool(name="xa", bufs=2) as xap,
            tc.tile_pool(name="xT", bufs=1) as xtp,
            tc.tile_pool(name="pb", bufs=4) as pbp,
            tc.tile_pool(name="nrm", bufs=2) as nrm,
            tc.tile_pool(name="oc", bufs=3) as ocp,
            tc.tile_pool(name="qkvps", bufs=2, space="PSUM") as qps,
            tc.tile_pool(name="sps", bufs=2, space="PSUM") as sps,
            tc.tile_pool(name="ops", bufs=1, space="PSUM") as ops,
        ):
            # --- constants, then weights / x loads, spread over 4 DMA queues ---------------
            # --- constants -------------------------------------------------
            ident = const.tile([P, P], DT)
            make_identity(nc, ident[:])
            identr = const.tile([P, P], DT)
            nc.vector.tensor_copy(_r(identr[:]), ident[:])
            ones_t = const.tile([P, P], DT)
            nc.gpsimd.memset(ones_t[:], 0.0)
            nc.vector.tensor_scalar_add(_r(ones_t[:]), ones_t[:], 1.0)

            bq_t = const.tile([P, FC], DT)
            bk_t = const.tile([P, FC], DT)
            bq8 = const.tile([P, FC], DT)
            bv_row = const.tile([P, FH], DT)



            kTt = big.tile([P, FC, T], DTB)             # K^T, bf16
            v66 = big.tile([P, NKT, NHC, HD + 1], DTB)  # V (+bias), ones col
            nc.gpsimd.memset(v66[:, :, :, HD:HD + 1], 1.0)


            wq_t = wpool.tile([P, CC, FH], DT, tag="wq")
            wk_t = wpool.tile([P, CC, FH], DT, tag="wk")
            wv_t = wpool.tile([P, CC, FH], DT, tag="wv")
            wp_t = wpool.tile([P, FC, C], DT, tag="wp")
            xas = {}

            def load_xa(tt, to, eng):
                xa = xap.tile([P, C], DT, tag="xa", name=f"xa_{tt}_{to}")
                rows = xb.ap()[tt * QT + to * P: tt * QT + (to + 1) * P, :]
                eng.dma_start(_r(xa[:]), _r(rows))
                xas[(tt, to)] = xa

            # startup: xa(0) split over scalar+sync so x^T(0) is ready
            # early; wq/wk split across both HWDGE queues while ACT is
            # still idle (dma_start occupies the ISSUING engine's
            # sequencer ~625ns). Everything later issues from sync (SP has
            # no compute); xa(tt) is prefetched one tile ahead from the
            # driver loop.
            # first token-block load split in column halves: transposes of
            # the first 4 c-chunks start ~1.5us earlier
            xa0 = xap.tile([P, C], DT, tag="xa", name="xa_0_0")
            nc.scalar.dma_start(_r(xa0[:, 0:QT]), _r(xb.ap()[0:P, 0:QT]))
            nc.sync.dma_start(_r(xa0[:, QT:C]), _r(xb.ap()[0:P, QT:C]))
            xas[(0, 0)] = xa0
            for to in range(1, 4):
                load_xa(0, to, nc.scalar if to < 2 else nc.sync)
            for cc in range(CC):
                eng = nc.sync if cc < 4 else nc.gpsimd
                eng.dma_start(_r(wq_t[:, cc, :]),
                              _r(wq.ap()[cc * P:(cc + 1) * P, :]))
            nc.sync.dma_start(bq_t[:], bq.ap().rearrange("(c p) -> p c", p=P))
            nc.sync.dma_start(bk_t[:], bk.ap().rearrange("(c p) -> p c", p=P))
            nc.sync.dma_start(bv_row[0:1, :],
                              bv.ap().rearrange("(a f) -> a f", a=1))
            nc.vector.tensor_scalar_mul(bq8[:], bq_t[:], 0.125)
            for cc in range(CC):
                eng = nc.sync if cc < 4 else nc.gpsimd
                eng.dma_start(_r(wk_t[:, cc, :]),
                              _r(wk.ap()[cc * P:(cc + 1) * P, :]))
            for to in range(4):
                load_xa(1, to, nc.scalar)
            for cc in range(CC):
                nc.sync.dma_start(_r(wv_t[:, cc, :]),
                                  _r(wv.ap()[cc * P:(cc + 1) * P, :]))
            for fc in range(FC):
                nc.sync.dma_start(_r(wp_t[:, fc, :]),
                                  _r(wp.ap()[fc * P:(fc + 1) * P, :]))

            # broadcast v-bias along tokens: vb_bc[t, f] = bv[f]
            vb_bc = const.tile([P, FH], DT)

            def make_vb():
                vb_ps = qps.tile([P, QT], DT, tag="qkv")
                nc.tensor.matmul(vb_ps[:], lhsT=ones_t[0:1, :],
                                 rhs=bv_row[0:1, :], start=True, stop=True)
                nc.vector.tensor_copy(vb_bc[:], vb_ps[:])

            xTs = {}

            def qkv_x(tt):
                """x^T for tokens [tt*512, (tt+1)*512)."""
                xT = xtp.tile([P, CC, QT], DT, tag="xT", name=f"xT_{tt}")
                xTs[tt] = xT
                for to in range(4):        # 128-token blocks
                    xa = xas[(tt, to)]
                    for g in range(2):     # c-chunk groups of 4
                        pst = qps.tile([P, QT], DT, tag="qkv",
                                       name=f"pst_{to}_{g}")
                        for cc4 in range(4):
                            cc = g * 4 + cc4
                            nc.tensor.transpose(
                                _r(pst[:, cc4 * P:(cc4 + 1) * P]),
                                _r(xa[:, cc * P:(cc + 1) * P]), _r(identr[:]))
                        dst = _r(xT[:, g * 4:g * 4 + 4, to * P:(to + 1) * P])
                        srcap = pst[:].rearrange("p (c t) -> p c t", t=P)
                        if tt == 0 and g == 1:
                            # ACT is idle at startup; halve the copy chain
                            nc.scalar.activation(
                                dst, _r(srcap),
                                mybir.ActivationFunctionType.Identity)
                        else:
                            nc.vector.tensor_copy(dst, srcap)

            def qkv_qk(tt, fc, qT):
                xT = xTs[tt]
                for wt, dst, bias_ap, scale in (
                    (wq_t, ("q", qT), bq8, 0.125),
                    (wk_t, ("k", None), bk_t, 1.0),
                ):
                    ps = qps.tile([P, QT], DT, tag="qkv")
                    for cc in range(CC):
                        nc.tensor.matmul(
                            ps[:],
                            lhsT=_r(wt[:, cc, fc * P:(fc + 1) * P]),
                            rhs=_r(xT[:, cc, :]),
                            start=(cc == 0), stop=(cc == CC - 1))
                    if dst[0] == "q":
                        nc.vector.tensor_scalar(
                            dst[1][:, fc, :], ps[:],
                            0.125, bias_ap[:, fc:fc + 1],
                            mybir.AluOpType.mult, mybir.AluOpType.add)
                    else:
                        nc.vector.tensor_scalar(
                            kTt[:, fc, tt * QT:(tt + 1) * QT], ps[:],
                            1.0, bias_ap[:, fc:fc + 1],
                            mybir.AluOpType.mult, mybir.AluOpType.add)

            def qkv_v_chunk(tt, ts):
                xT = xTs[tt]
                ps = qps.tile([P, FH], DT, tag="qkv")
                for cc in range(CC):
                    nc.tensor.matmul(
                        ps[:],
                        lhsT=_r(xT[:, cc, ts * P:(ts + 1) * P]),
                        rhs=_r(wv_t[:, cc, :]),
                        start=(cc == 0), stop=(cc == CC - 1))
                kt = tt * 4 + ts
                nc.vector.tensor_tensor(
                    out=v66[:, kt, :, 0:HD],
                    in0=ps[:].rearrange("p (h d) -> p h d", d=HD),
                    in1=vb_bc[:].rearrange("p (h d) -> p h d", d=HD),
                    op=mybir.AluOpType.add)

            def qkv_v(tt):
                for ts in range(4):
                    qkv_v_chunk(tt, ts)

            def attn_pair(i, j, qT, atn, fillers=None):
                """Heads (2i, 2i+1) for q-tile j; diagonal blocks first.

                Both heads' scores live in one [P, 1024] psum (2 banks, one
                per head) so exp runs as a single double-width ACT op.
                `fillers` is a deque of thunks emitting independent PE work;
                one is spliced in after each k-tile so the in-order PE
                stream has something to chew on while exp paces the S slots.
                """
                heads = ((2 * i, 0), (2 * i + 1, HD))
                O = ops.tile([P, 2 * QT], DT, tag="O", name=f"O_{i}_{j}")
                nk = 4 * j + 4
                kts = ([4 * j, 4 * j + 1, 4 * j + 2, 4 * j + 3]
                       + list(range(4 * j)))
                for idx, kt in enumerate(kts):
                    delta = max(0, kt * P - j * QT)
                    S = sps.tile([P, 2 * QT], DT, tag="S", name=f"S_{i}_{kt}")
                    for hh, (h, pb0) in enumerate(heads):
                        nc.tensor.matmul(
                            S[:, hh * QT + delta:(hh + 1) * QT],
                            lhsT=kTt[pb0:pb0 + HD, i, kt * P:(kt + 1) * P],
                            rhs=qT[pb0:pb0 + HD, i, delta:QT],
                            start=True, stop=True)
                    Pt = pbp.tile([P, 2 * QT], DTB, tag="P")
                    nc.scalar.activation(
                        Pt[:].rearrange("p (h w) -> p h w", h=2)[:, :, delta:QT],
                        S[:].rearrange("p (h w) -> p h w", h=2)[:, :, delta:QT],
                        Exp)
                    if kt >= 4 * j:
                        # causal mask of the diagonal 128-block, both heads
                        nc.gpsimd.affine_select(
                            out=Pt[:].rearrange("p (h w) -> p h w", h=2)
                                [:, :, delta:delta + P],
                            in_=Pt[:].rearrange("p (h w) -> p h w", h=2)
                                [:, :, delta:delta + P],
                            pattern=[[0, 2], [1, P]],
                            compare_op=mybir.AluOpType.is_gt, fill=0.0,
                            base=1, channel_multiplier=-1)
                    for hh, (h, pb0) in enumerate(heads):
                        nc.tensor.matmul(
                            O[0:HD + 1, hh * QT + delta:(hh + 1) * QT],
                            lhsT=v66[:, kt, h, :],
                            rhs=Pt[:, hh * QT + delta:(hh + 1) * QT],
                            start=(idx == 0), stop=(idx == nk - 1))
                # drain O psum: reciprocals straight from psum + one copy;
                # then ONE gpsimd partition-broadcast of both recip rows and
                # all-SBUF multiplies. No psum is held by normalization, so
                # the next pair's S tiles are never blocked (the old bc
                # matmul held an S slot until its DVE mult drained).
                ocp_t = nrm.tile([P, 2 * QT], DT, tag="Ocpy")
                rb0 = nrm.tile([1, 2 * QT], DT, tag="rb0")
                rbc = nrm.tile([P, 2 * QT], DT, tag="rbc")
                # reciprocal of the denom row (psum -> sbuf, partition 64),
                # then DMA it to partition 0: engines cannot shift
                # partitions, and partition_broadcast reads the tile's
                # partition 0 regardless of the source AP offset
                with nc.allow_low_precision(reason="f32r rounding of softmax denom reciprocal"):
                    nc.vector.reciprocal(_r(ocp_t[HD:HD + 1, :]),
                                         O[HD:HD + 1, :])
                nc.vector.tensor_copy(_r(ocp_t[0:HD, :]), O[0:HD, :])
                nc.sync.dma_start(_r(rb0[0:1, :]), _r(ocp_t[HD:HD + 1, :]))
                nc.gpsimd.partition_broadcast(rbc[0:HD, :], rb0[0:1, :])
                nc.vector.tensor_tensor(
                    out=_r(atn[0:HD, i, :]), in0=ocp_t[0:HD, 0:QT],
                    in1=rbc[0:HD, 0:QT], op=mybir.AluOpType.mult)
                # head 2: multiply in place and bounce the half via DMA
                nc.vector.tensor_tensor(
                    out=_r(ocp_t[0:HD, QT:2 * QT]), in0=ocp_t[0:HD, QT:2 * QT],
                    in1=rbc[0:HD, QT:2 * QT], op=mybir.AluOpType.mult)
                nc.sync.dma_start(_r(atn[HD:2 * HD, i, :]),
                                  _r(ocp_t[0:HD, QT:2 * QT]))

            def proj_half(tt, atn, t4, co):
                """Output projection for token block tt*4 + t4, col half co."""
                t_ = tt * 4 + t4
                pp = sps.tile([P, 2 * QT], DT, tag="S",
                              name=f"pp_{t_}_{co}")
                for fc in range(FC):
                    nc.tensor.matmul(
                        pp[:, 0:QT],
                        lhsT=_r(atn[:, fc, t4 * P:(t4 + 1) * P]),
                        rhs=_r(wp_t[:, fc, co * QT:(co + 1) * QT]),
                        start=(fc == 0), stop=(fc == FC - 1))
                oc = ocp.tile([P, QT], DT, tag="oc")
                if tt == NQT - 1:
                    # ACT is idle after the last exp; keep the tail drain off
                    # the DVE queue
                    nc.scalar.activation(_r(oc[:]), _r(pp[:, 0:QT]),
                                         mybir.ActivationFunctionType.Identity)
                else:
                    nc.vector.tensor_copy(oc[:], pp[:, 0:QT])
                nc.sync.dma_start(
                    yp.ap()[t_ * P:(t_ + 1) * P, co * QT:(co + 1) * QT],
                    oc[:])

            def proj_piece(tt, atn, t4):
                for co in range(2):
                    proj_half(tt, atn, t4, co)

            # --- driver: tight interleave --------------------------------
            # per tt: qkv_x, qk(fc0), v; attention pairs interleaved with
            # qk(fc+1) and the previous tile's projection pieces so the PE
            # stream always has independent work while exp paces the S slots.
            prev_atn = None
            for tt in range(NQT):
                qT = qTp.tile([P, FC, QT], DTB, tag="qT", name=f"qT_{tt}")
                atn = atp.tile([P, FC, QT], DT, tag="atn", name=f"atn_{tt}")
                qkv_x(tt)
                qkv_qk(tt, 0, qT)
                if tt == 0:
                    make_vb()
                qkv_v(tt)
                if tt + 2 < NQT:
                    for to in range(4):
                        load_xa(tt + 2, to, nc.scalar)
                for i in range(4):
                    attn_pair(i, j=tt, qT=qT, atn=atn)
                    if i + 1 < 4:
                        qkv_qk(tt, i + 1, qT)
                    if tt > 0:
                        proj_piece(tt - 1, prev_atn, i)
                prev_atn = atn
            for t4 in range(4):
                proj_piece(NQT - 1, prev_atn, t4)

    nc.finalize()
    return nc


# ---------------------------------------------------------------------------
# host side: cached jitted SPMD runner
# ---------------------------------------------------------------------------

_RUNNER = None


def _make_runner():
    import jax
    import jax.numpy as jnp
    from jax.experimental.shard_map import shard_map
    from jax.sharding import Mesh, PartitionSpec

    from concourse import bass2jax

    nc = build_nc()
    bass2jax.install_neuronx_cc_hook()

    partition_name = (nc.partition_id_tensor.name
                      if nc.partition_id_tensor else None)
    in_names = []
    out_names = []
    out_avals = []
    out_shapes = []
    for alloc in nc.m.functions[0].allocations:
        if not isinstance(alloc, mybir.MemoryLocationSet):
            continue
        name = alloc.memorylocations[0].name
        if alloc.kind == "ExternalInput":
            if name != partition_name:
                in_names.append(name)
        elif alloc.kind == "ExternalOutput":
            shape = tuple(alloc.tensor_shape)
            dtype = mybir.dt.np(alloc.dtype)
            out_avals.append(jax.core.ShapedArray(shape, dtype))
            out_shapes.append((name, shape, dtype))
            out_names.append(name)
    n_params = len(in_names)
    n_outs = len(out_avals)
    all_in_names = list(in_names) + list(out_names)
    if partition_name is not None:
        all_in_names.append(partition_name)
    donate = tuple(range(n_params, n_params + n_outs))

    def _body(*args):
        operands = list(args)
        if partition_name is not None:
            operands.append(bass2jax.partition_id_tensor())
        outs = bass2jax._bass_exec_p.bind(
            *operands,
            out_avals=tuple(out_avals),
            in_names=tuple(all_in_names),
            out_names=tuple(out_names),
            lowering_input_output_aliases=(),
            sim_require_finite=True,
            sim_require_nnan=True,
            nc=nc,
        )
        return tuple(outs)

    devices = jax.devices()[:NCORES]
    mesh = Mesh(np.asarray(devices), ("core",))
    in_specs = (PartitionSpec("core"),) * (n_params + n_outs)
    out_specs = (PartitionSpec("core"),) * n_outs
    sharded = jax.jit(
        shard_map(_body, mesh=mesh, in_specs=in_specs, out_specs=out_specs,
                  check_rep=False),
        donate_argnums=donate, keep_unused=True)

    def run(in_maps):
        concat_in = [
            np.concatenate([np.asarray(in_maps[c][name]) for c in range(NCORES)],
                           axis=0)
            for name in in_names
        ]
        concat_zeros = [
            np.zeros((NCORES * s[0], *s[1:]), dt) for (_, s, dt) in out_shapes
        ]
        out_arrs = sharded(*concat_in, *concat_zeros)
        return [
            {name: np.asarray(out_arrs[i]).reshape(NCORES, *shape)[c]
             for i, (name, shape, _) in enumerate(out_shapes)}
            for c in range(NCORES)
        ]

    return run, sharded, in_names, out_shapes, mesh


def _get_runner():
    global _RUNNER
    if _RUNNER is None:
        _RUNNER = _make_runner()
    return _RUNNER


def _in_maps(x, w_attn, b_attn, w_proj):
    maps = []
    for c in range(NCORES):
        b, j = c // 2, c % 2
        f0 = FH * j
        maps.append({
            "xb": np.ascontiguousarray(x[b]),
            "wq": np.ascontiguousarray(w_attn[:, f0:f0 + FH]),
            "wk": np.ascontiguousarray(w_attn[:, C + f0:C + f0 + FH]),
            "wv": np.ascontiguousarray(w_attn[:, 2 * C + f0:2 * C + f0 + FH]),
            "wp": np.ascontiguousarray(w_proj[f0:f0 + FH, :]),
            "bq": np.ascontiguousarray(b_attn[f0:f0 + FH]),
            "bk": np.ascontiguousarray(b_attn[C + f0:C + f0 + FH]),
            "bv": np.ascontiguousarray(b_attn[2 * C + f0:2 * C + f0 + FH]),
        })
    return maps


def kernel(x, w_attn, b_attn, w_proj, b_proj):
    x = np.asarray(x, dtype=np.float32)
    w_attn = np.asarray(w_attn, dtype=np.float32)
    b_attn = np.asarray(b_attn, dtype=np.float32)
    w_proj = np.asarray(w_proj, dtype=np.float32)
    b_proj = np.asarray(b_proj, dtype=np.float32)

    run, *_ = _get_runner()
    results = run(_in_maps(x, w_attn, b_attn, w_proj))

    y = np.empty((B, T, C), dtype=np.float32)
    for b in range(B):
        y[b] = results[2 * b]["yp"] + results[2 * b + 1]["yp"] + b_proj
    return y
